# revision 16
# baseline (speedup 1.0000x reference)
"""Trainium2 Bass kernel for YOLO-style DetectionLayer decode.

Full input  x: (16, 255, 76, 76) f32  (channel-major: 3 anchors x 85 ch)
Full output  : (16, 17328, 85) f32   (position-major: 3*76*76 rows x 85 ch)

Math per (b, a, gy, gx):
  out[..., 0] = (sigmoid(tx) + gx) * 8
  out[..., 1] = (sigmoid(ty) + gy) * 8
  out[..., 2] = exp(tw) * ANCHOR[a][0]        (stride cancels)
  out[..., 3] = exp(th) * ANCHOR[a][1]
  out[..., 4:] = sigmoid(...)

Sharding: pure data-parallel over batch: 2 batches per core x 8 cores.

Per-core kernel:
  - HW constraint (measured): SBUF DMA writes covering all 128 partitions
    run at ~857ns per 23KB packet; ANY partial-partition write runs at
    ~2x that.  So the 510 input channel rows load as FOUR full-128-row
    tiles at rows [0:128], [128:256], [255:383], [382:510] (2 duplicate
    rows) -- minimal traffic at full rate -- spread across the sync,
    scalar (both HWDGE) and gpsimd (SWDGE) queues so descriptor
    generation is never the pacer and the streams start early.
  - TensorE transposes 46 chunks per (b, a) pair; chunk j takes
    positions {45 p + j} so output partition p holds 45 consecutive
    output rows -> 15.3KB contiguous store runs.  Transpose mode is
    pure routing (needs a square permutation selector; junk rows land
    in junk columns).  Three per-pair cases by where the 85 channels
    sit in the tiles:
      * rows 0..84 of one tile: 85-partition operands + 85x85 identity;
        output is exactly 85 cols -> 6 chunks per PSUM bank.
      * rows b..b+84 of one tile (b=42/43): 128-partition operands +
        square perm (row b+c -> col c); 128-col writes at 85-col stride
        overwrite each other's junk -> 5 chunks per bank, still
        contiguous for evacuation.
      * split across two tiles: two transposes per chunk into two PSUM
        banks (piece A: rows 85.. -> cols 0..42, piece B -> cols
        43..84), packed at 43/42-col stride -> 9 chunks per bank pair.
  - ScalarE evacuates each bank with fused tanh(v/2) (sigmoid =
    .5+.5*tanh; one ACT table set holds both tanh and exp), plus true
    Exp on the w/h cols straight from PSUM raw values.
  - VectorE: whole-tile affine .5*t+.5 (2x port mode) turns tanh into
    sigmoid; x/y = 8*s + 8*grid (host table); w/h = (2A)*v - A
    (compensating the affine on the exp'd cols).
  - Main stores ride the sync HWDGE queue; the six 16-position tails
    accumulate in one SBUF tile and go out in a single strided store.
"""

import os
import sys

import numpy as np

for _p in ("/opt/trn_rl_repo", "/root/.axon_site/_ro/trn_rl_repo"):
    if os.path.isdir(_p) and _p not in sys.path:
        sys.path.append(_p)

import concourse.bacc as bacc
import concourse.bass as bass
import concourse.mybir as mybir
import concourse.tile as tile
from concourse.bass_utils import run_bass_kernel_spmd

ANCHORS = np.array([[10.0, 13.0], [16.0, 30.0], [33.0, 23.0]], dtype=np.float32)
NB_FULL = 16
N_CORES = 8
NB = NB_FULL // N_CORES  # batches per core
NA = 3
NC = 85  # 5 + 80 channels
NG = 76
NPOS = NG * NG  # 5776
STRIDE = 8.0
NPAIR = NB * NA  # 6

# Position-chunking: output partition p holds rows [45p, 45p+45); chunk j
# gathers positions {45p + j}. 5776 = 128*45 + 16 -> 16-row tail.
RPP = 45  # rows per partition (main part)
MAIN = 128 * RPP  # 5760
TAIL = NPOS - MAIN  # 16

# input tiles: full-128-row loads covering the 510 channel rows
TILE_ROWS = [(0, 128), (128, 256), (255, 383), (382, 510)]
# pair -> how its 85 channels sit in the tiles:
#   ("one", tile, shift): channels at tile rows shift..shift+84
#   ("two", tileA, shiftA, nA, tileB, shiftB): first nA channels at
#       tileA rows shiftA.., rest at tileB rows shiftB..
PAIR_SRC = [
    ("one", 0, 0),
    ("two", 0, 85, 43, 1, 0),
    ("one", 1, 42),
    ("one", 2, 0),
    ("two", 2, 85, 43, 3, 1),
    ("one", 3, 43),
]

F32 = mybir.dt.float32
AF = mybir.ActivationFunctionType
OP = mybir.AluOpType


def _groups(cpb):
    return [(g * cpb, min(cpb, RPP - g * cpb)) for g in range(-(-RPP // cpb))]


def _perm_for(shift, n=NC):
    """Square 128x128 permutation: row shift+c -> col c for c<n, junk
    rows -> junk cols (any bijection)."""
    p = np.zeros((128, 128), dtype=np.float32)
    rows = (shift + np.arange(n)) % 128
    p[rows, np.arange(n)] = 1.0
    junk_rows = [r for r in range(128) if r not in set(rows.tolist())]
    junk_cols = [c for c in range(n, 128)]
    assert len(junk_rows) == len(junk_cols)
    p[junk_rows, junk_cols] = 1.0
    return p


def _perm_range(shift, c0, n):
    """Square perm: row shift+k -> col c0+k for k<n, junk -> junk."""
    p = np.zeros((128, 128), dtype=np.float32)
    rows = shift + np.arange(n)
    cols = c0 + np.arange(n)
    p[rows, cols] = 1.0
    junk_rows = [r for r in range(128) if r not in set(rows.tolist())]
    junk_cols = [c for c in range(128) if c not in set(cols.tolist())]
    p[junk_rows, junk_cols] = 1.0
    return p


def _tables():
    p = np.arange(128)[:, None]
    j = np.arange(RPP)[None, :]
    r = p * RPP + j
    gg = np.empty((128, 2 * RPP), dtype=np.float32)
    gg[:, 0::2] = (r % NG) * STRIDE
    gg[:, 1::2] = (r // NG) * STRIDE
    rt = MAIN + np.arange(TAIL)
    gxt = ((rt % NG) * STRIDE).astype(np.float32)[:, None]
    gyt = float((MAIN // NG) * STRIDE)  # rows 5760..5775 all have gy=75
    assert np.all(rt // NG == MAIN // NG)
    ident0 = np.eye(NC, dtype=np.float32)
    # stacked square perms: [selA, selB1, selB4, sel42, sel43].  Both
    # split pieces route their real channels to HEAD columns (piece A:
    # cols 0..42, piece B: cols 0..41): with overwrite packing, a later
    # write's junk may only land where junk already was, so real data
    # must sit at the start of each write's column window.
    sels = np.stack(
        [
            _perm_range(85, 0, 43),  # piece A (pairs 1 and 4)
            _perm_range(0, 0, 42),  # piece B pair 1 (t1 rows 0..41)
            _perm_range(1, 0, 42),  # piece B pair 4 (t3 rows 1..42)
            _perm_for(42),  # pair 2
            _perm_for(43),  # pair 5
        ]
    )
    return gg, gxt, gyt, ident0, sels


GG_TABLE, GXT_TABLE, GYT_CONST, IDENT0_TABLE, SELS_TABLE = _tables()
SEL_A, SEL_B1, SEL_B4, SEL_42, SEL_43 = range(5)


def build_program():
    nc = bacc.Bacc(None, target_bir_lowering=False)

    x = nc.dram_tensor("x", (NB, NA * NC, NG, NG), F32, kind="ExternalInput")
    out = nc.dram_tensor("out", (NB, NA * NPOS, NC), F32, kind="ExternalOutput")
    gg = nc.dram_tensor("gg", (128, 2 * RPP), F32, kind="ExternalInput")
    gxt = nc.dram_tensor("gxt", (TAIL, 1), F32, kind="ExternalInput")
    id0 = nc.dram_tensor("id0", (NC, NC), F32, kind="ExternalInput")
    sels = nc.dram_tensor("sels", (5, 128, 128), F32, kind="ExternalInput")

    with tile.TileContext(nc) as tc:
        with (
            tc.tile_pool(name="constp", bufs=1) as constp,
            tc.tile_pool(name="xp", bufs=1) as xp,
            tc.tile_pool(name="outp", bufs=3) as outp,
            tc.tile_pool(name="pp", bufs=4, space="PSUM") as pp,
            tc.tile_pool(name="tp", bufs=2, space="PSUM") as tp,
        ):
            # small constants first on the sync queue (a few packets)
            id0s = constp.tile([NC, NC], F32)
            nc.sync.dma_start(out=id0s[:], in_=id0[:])
            selss = constp.tile([128, 5 * 128], F32)
            nc.sync.dma_start(
                out=selss[:].rearrange("p (s c) -> p s c", s=5),
                in_=sels.rearrange("s p c -> p s c"),
            )
            ggs = constp.tile([128, 2 * RPP], F32)
            nc.sync.dma_start(out=ggs[:], in_=gg[:])
            gxts = constp.tile([TAIL, 1], F32)
            nc.sync.dma_start(out=gxts[:], in_=gxt[:])
            ggv = ggs.rearrange("p (k c) -> p k c", c=2)

            def sel(i):
                return selss[:, i * 128 : (i + 1) * 128]

            xf = x.rearrange("b c h w -> (b c) (h w)")

            # four full-128-row tiles across three DMA queues
            xts = [xp.tile([128, NPOS], F32, name=f"xt{i}") for i in range(4)]
            load_eng = [nc.sync, nc.gpsimd, nc.scalar, nc.gpsimd]
            for i, (r0, r1) in enumerate(TILE_ROWS):
                load_eng[i].dma_start(
                    out=xts[i][0 : r1 - r0, :], in_=xf[r0:r1, :]
                )

            # all six 16-position tails accumulate here; one store at the end
            tta = constp.tile([TAIL, 512], F32)

            # (tile, 45, 128) chunk views: [:, j, :] = chunk j
            def chunks(t, np_):
                return xts[t][0:np_, 0:MAIN].rearrange("c (m j) -> c j m", j=RPP)

            for pair in range(NPAIR):
                b, a = divmod(pair, NA)
                aw = float(ANCHORS[a, 0])
                ah = float(ANCHORS[a, 1])
                src = PAIR_SRC[pair]
                ot = outp.tile([128, RPP * NC + 1], F32, tag="ot")
                otr = ot[:, 0 : RPP * NC].rearrange("p (k c) -> p k c", c=NC)
                tt = tta[:, pair * NC : (pair + 1) * NC]
                pst = tp.tile([TAIL, 512], F32, tag="pst")

                if src[0] == "one" and src[2] == 0:
                    # 85-partition operands, 85x85 identity, 6 chunks/bank
                    t = src[1]
                    xm = chunks(t, NC)
                    for k0, nk in _groups(6):
                        ps = pp.tile([128, 512], F32, tag="ps")
                        for m in range(nk):
                            nc.tensor.transpose(
                                ps[:, NC * m : NC * (m + 1)],
                                xm[:, k0 + m, :],
                                id0s[:, :],
                                tile_position=(0, 0),
                            )
                        nc.scalar.activation(
                            ot[:, k0 * NC : (k0 + nk) * NC],
                            ps[:, 0 : nk * NC],
                            AF.Tanh,
                            scale=0.5,
                        )
                        psv = ps[:, 0 : nk * NC].rearrange("p (k c) -> p k c", c=NC)
                        nc.scalar.activation(
                            otr[:, k0 : k0 + nk, 2:4], psv[:, :, 2:4], AF.Exp
                        )
                    nc.tensor.transpose(
                        pst[:, 0:NC],
                        xts[t][0:NC, MAIN:NPOS],
                        id0s[:, :],
                        tile_position=(0, 0),
                    )
                    nc.scalar.activation(tt, pst[:, 0:NC], AF.Tanh, scale=0.5)
                    nc.scalar.activation(tt[:, 2:4], pst[:, 2:4], AF.Exp)

                elif src[0] == "one":
                    # 128-partition operands + square perm; 128-col writes
                    # at 85-col stride (overwrite packing), 5 chunks/bank
                    t, shift = src[1], src[2]
                    sq = sel(SEL_42 if shift == 42 else SEL_43)
                    xm = chunks(t, 128)
                    for k0, nk in _groups(5):
                        ps = pp.tile([128, 512], F32, tag="ps")
                        for m in range(nk):
                            nc.tensor.transpose(
                                ps[:, NC * m : NC * m + 128],
                                xm[:, k0 + m, :],
                                sq,
                                tile_position=(0, 0),
                            )
                        nc.scalar.activation(
                            ot[:, k0 * NC : (k0 + nk) * NC],
                            ps[:, 0 : nk * NC],
                            AF.Tanh,
                            scale=0.5,
                        )
                        psv = ps[:, 0 : nk * NC].rearrange("p (k c) -> p k c", c=NC)
                        nc.scalar.activation(
                            otr[:, k0 : k0 + nk, 2:4], psv[:, :, 2:4], AF.Exp
                        )
                    nc.tensor.transpose(
                        pst[:, 0:128],
                        xts[t][0:128, MAIN:NPOS],
                        sq,
                        tile_position=(0, 0),
                    )
                    nc.scalar.activation(tt, pst[:, 0:NC], AF.Tanh, scale=0.5)
                    nc.scalar.activation(tt[:, 2:4], pst[:, 2:4], AF.Exp)

                else:
                    # split across two tiles: piece A -> cols 0..42 (43-col
                    # stride), piece B -> cols 43..84 (42-col stride), both
                    # 9 chunks/bank with overwrite packing
                    _, tA, sA, nA, tB, sB = src
                    selA = sel(SEL_A)
                    selB = sel(SEL_B1 if pair == 1 else SEL_B4)
                    xmA = chunks(tA, 128)
                    xmB = chunks(tB, 128)
                    for k0, nk in _groups(9):
                        psA = pp.tile([128, 512], F32, tag="ps")
                        psB = pp.tile([128, 512], F32, tag="ps")
                        for m in range(nk):
                            nc.tensor.transpose(
                                psA[:, nA * m : nA * m + 128],
                                xmA[:, k0 + m, :],
                                selA,
                                tile_position=(0, 0),
                            )
                            nc.tensor.transpose(
                                psB[:, (NC - nA) * m : (NC - nA) * m + 128],
                                xmB[:, k0 + m, :],
                                selB,
                                tile_position=(0, 0),
                            )
                        pvA = psA[:, 0 : nk * nA].rearrange("p (k c) -> p k c", c=nA)
                        pvB = psB[:, 0 : nk * (NC - nA)].rearrange(
                            "p (k c) -> p k c", c=NC - nA
                        )
                        nc.scalar.activation(
                            otr[:, k0 : k0 + nk, 0:nA], pvA, AF.Tanh, scale=0.5
                        )
                        nc.scalar.activation(
                            otr[:, k0 : k0 + nk, nA:NC], pvB, AF.Tanh, scale=0.5
                        )
                        nc.scalar.activation(
                            otr[:, k0 : k0 + nk, 2:4], pvA[:, :, 2:4], AF.Exp
                        )
                    pstB = tp.tile([TAIL, 512], F32, tag="pst")
                    nc.tensor.transpose(
                        pst[:, 0:128],
                        xts[tA][0:128, MAIN:NPOS],
                        selA,
                        tile_position=(0, 0),
                    )
                    nc.tensor.transpose(
                        pstB[:, 0:128],
                        xts[tB][0:128, MAIN:NPOS],
                        selB,
                        tile_position=(0, 0),
                    )
                    nc.scalar.activation(
                        tt[:, 0:nA], pst[:, 0:nA], AF.Tanh, scale=0.5
                    )
                    nc.scalar.activation(
                        tt[:, nA:NC], pstB[:, 0 : NC - nA], AF.Tanh, scale=0.5
                    )
                    nc.scalar.activation(tt[:, 2:4], pst[:, 2:4], AF.Exp)

                # VectorE fixups (main): whole-tile affine at 2x port mode
                # (needs an even element count -> one memset pad column),
                # then per-channel-type corrections.
                nc.vector.memset(ot[:, RPP * NC : RPP * NC + 1], 0.0)
                nc.vector.tensor_scalar(
                    ot[:, 0 : RPP * NC + 1],
                    ot[:, 0 : RPP * NC + 1],
                    0.5,
                    0.5,
                    OP.mult,
                    OP.add,
                )
                xy = otr[:, :, 0:2]
                nc.vector.tensor_scalar(xy, xy, STRIDE, None, OP.mult)
                nc.vector.tensor_tensor(xy, xy, ggv, OP.add)
                wv = otr[:, :, 2:3]
                nc.vector.tensor_scalar(wv, wv, 2.0 * aw, -aw, OP.mult, OP.add)
                hv = otr[:, :, 3:4]
                nc.vector.tensor_scalar(hv, hv, 2.0 * ah, -ah, OP.mult, OP.add)

                # VectorE fixups (tail); odd count (85) -> 84 + last col
                nc.vector.tensor_scalar(
                    tt[:, 0:84], tt[:, 0:84], 0.5, 0.5, OP.mult, OP.add
                )
                nc.vector.tensor_scalar(
                    tt[:, 84:85], tt[:, 84:85], 0.5, 0.5, OP.mult, OP.add
                )
                nc.vector.tensor_scalar(
                    tt[:, 0:1], tt[:, 0:1], STRIDE, gxts[:], OP.mult, OP.add
                )
                nc.vector.tensor_scalar(
                    tt[:, 1:2], tt[:, 1:2], STRIDE, GYT_CONST, OP.mult, OP.add
                )
                nc.vector.tensor_scalar(
                    tt[:, 2:3], tt[:, 2:3], 2.0 * aw, -aw, OP.mult, OP.add
                )
                nc.vector.tensor_scalar(
                    tt[:, 3:4], tt[:, 3:4], 2.0 * ah, -ah, OP.mult, OP.add
                )

                # main store on the sync HWDGE queue: 128 runs of 15.3KB
                obase = a * NPOS
                nc.sync.dma_start(
                    out=out[b, obase : obase + MAIN, :].rearrange(
                        "(p j) c -> p (j c)", p=128
                    ),
                    in_=ot[:, 0 : RPP * NC],
                )

            # one combined tail store: out[b, a*NPOS + 5760 + t, c] with
            # partition t and free (b, a, c) = tta col (b*3+a)*85 + c
            tails = out.rearrange("b (a q) c -> q b a c", a=NA)
            nc.sync.dma_start(
                out=tails[MAIN:NPOS],
                in_=tta[:, 0 : NPAIR * NC].rearrange(
                    "t (b a c) -> t b a c", b=NB, a=NA
                ),
            )

    nc.compile()
    return nc


_NC_CACHE = None


def _get_program():
    global _NC_CACHE
    if _NC_CACHE is None:
        _NC_CACHE = build_program()
    return _NC_CACHE


def run(x, trace=False, **kwargs):
    """x: full (16, 255, 76, 76) f32. Returns (full_out, BassKernelResults)."""
    x = np.ascontiguousarray(np.asarray(x, dtype=np.float32))
    assert x.shape == (NB_FULL, NA * NC, NG, NG), x.shape
    nc = _get_program()
    in_maps = [
        {
            "x": np.ascontiguousarray(x[c * NB : (c + 1) * NB]),
            "gg": GG_TABLE,
            "gxt": GXT_TABLE,
            "id0": IDENT0_TABLE,
            "sels": SELS_TABLE,
        }
        for c in range(N_CORES)
    ]
    res = run_bass_kernel_spmd(nc, in_maps, list(range(N_CORES)), trace=trace, **kwargs)
    out = np.concatenate([res.results[c]["out"] for c in range(N_CORES)], axis=0)
    return out, res


def kernel(x):
    out, _ = run(x, trace=False)
    return out


# revision 19
# speedup vs baseline: 1.2060x; 1.2060x over previous
"""Trainium2 Bass kernel for YOLO-style DetectionLayer decode.

Full input  x: (16, 255, 76, 76) f32  (channel-major: 3 anchors x 85 ch)
Full output  : (16, 17328, 85) f32   (position-major: 3*76*76 rows x 85 ch)

Math per (b, a, gy, gx):
  out[..., 0] = (sigmoid(tx) + gx) * 8
  out[..., 1] = (sigmoid(ty) + gy) * 8
  out[..., 2] = exp(tw) * ANCHOR[a][0]        (stride cancels)
  out[..., 3] = exp(th) * ANCHOR[a][1]
  out[..., 4:] = sigmoid(...)

Sharding: pure data-parallel over batch: 2 batches per core x 8 cores.

Per-core kernel:
  - HW constraint (measured): SBUF DMA writes covering all 128 partitions
    run at ~857ns per 23KB packet; ANY partial-partition write runs at
    ~2x that.  So the 510 input channel rows load as FOUR full-128-row
    tiles at rows [0:128], [128:256], [255:383], [382:510] (2 duplicate
    rows) -- minimal traffic at full rate -- spread across the sync,
    scalar (both HWDGE) and gpsimd (SWDGE) queues so descriptor
    generation is never the pacer and the streams start early.
  - TensorE transposes 46 chunks per (b, a) pair; chunk j takes
    positions {45 p + j} so output partition p holds 45 consecutive
    output rows -> 15.3KB contiguous store runs.  Transpose mode is
    pure routing (needs a square permutation selector; junk rows land
    in junk columns).  Three per-pair cases by where the 85 channels
    sit in the tiles:
      * rows 0..84 of one tile: 85-partition operands + 85x85 identity;
        output is exactly 85 cols -> 6 chunks per PSUM bank.
      * rows b..b+84 of one tile (b=42/43): 128-partition operands +
        square perm (row b+c -> col c); 128-col writes at 85-col stride
        overwrite each other's junk -> 5 chunks per bank, still
        contiguous for evacuation.
      * split across two tiles: two transposes per chunk into two PSUM
        banks (piece A: rows 85.. -> cols 0..42, piece B -> cols
        43..84), packed at 43/42-col stride -> 9 chunks per bank pair.
  - ScalarE evacuates each bank with fused tanh(v/2) (sigmoid =
    .5+.5*tanh; one ACT table set holds both tanh and exp), plus true
    Exp on the w/h cols straight from PSUM raw values.
  - VectorE: whole-tile affine .5*t+.5 (2x port mode) turns tanh into
    sigmoid; x/y = 8*s + 8*grid (host table); w/h = (2A)*v - A
    (compensating the affine on the exp'd cols).
  - Main stores ride the sync HWDGE queue; the six 16-position tails
    accumulate in one SBUF tile and go out in a single strided store.
"""

import os
import sys

import numpy as np

for _p in ("/opt/trn_rl_repo", "/root/.axon_site/_ro/trn_rl_repo"):
    if os.path.isdir(_p) and _p not in sys.path:
        sys.path.append(_p)

import concourse.bacc as bacc
import concourse.bass as bass
import concourse.mybir as mybir
import concourse.tile as tile
from concourse.bass_utils import run_bass_kernel_spmd

ANCHORS = np.array([[10.0, 13.0], [16.0, 30.0], [33.0, 23.0]], dtype=np.float32)
NB_FULL = 16
N_CORES = 8
NB = NB_FULL // N_CORES  # batches per core
NA = 3
NC = 85  # 5 + 80 channels
NG = 76
NPOS = NG * NG  # 5776
STRIDE = 8.0
NPAIR = NB * NA  # 6

# Position-chunking: output partition p holds rows [45p, 45p+45); chunk j
# gathers positions {45p + j}. 5776 = 128*45 + 16 -> 16-row tail.
RPP = 45  # rows per partition (main part)
MAIN = 128 * RPP  # 5760
TAIL = NPOS - MAIN  # 16

# input tiles: full-128-row loads covering the 510 channel rows
TILE_ROWS = [(0, 128), (128, 256), (255, 383), (382, 510)]
# pair -> how its 85 channels sit in the tiles:
#   ("one", tile, shift): channels at tile rows shift..shift+84
#   ("two", tileA, shiftA, nA, tileB, shiftB): first nA channels at
#       tileA rows shiftA.., rest at tileB rows shiftB..
PAIR_SRC = [
    ("one", 0, 0),
    ("two", 0, 85, 43, 1, 0),
    ("one", 1, 42),
    ("one", 2, 0),
    ("two", 2, 85, 43, 3, 1),
    ("one", 3, 43),
]

F32 = mybir.dt.float32
AF = mybir.ActivationFunctionType
OP = mybir.AluOpType


def _groups(cpb):
    return [(g * cpb, min(cpb, RPP - g * cpb)) for g in range(-(-RPP // cpb))]


def _perm_for(shift, n=NC):
    """Square 128x128 permutation: row shift+c -> col c for c<n, junk
    rows -> junk cols (any bijection)."""
    p = np.zeros((128, 128), dtype=np.float32)
    rows = (shift + np.arange(n)) % 128
    p[rows, np.arange(n)] = 1.0
    junk_rows = [r for r in range(128) if r not in set(rows.tolist())]
    junk_cols = [c for c in range(n, 128)]
    assert len(junk_rows) == len(junk_cols)
    p[junk_rows, junk_cols] = 1.0
    return p


def _perm_range(shift, c0, n):
    """Square perm: row shift+k -> col c0+k for k<n, junk -> junk."""
    p = np.zeros((128, 128), dtype=np.float32)
    rows = shift + np.arange(n)
    cols = c0 + np.arange(n)
    p[rows, cols] = 1.0
    junk_rows = [r for r in range(128) if r not in set(rows.tolist())]
    junk_cols = [c for c in range(128) if c not in set(cols.tolist())]
    p[junk_rows, junk_cols] = 1.0
    return p


def _tables():
    p = np.arange(128)[:, None]
    j = np.arange(RPP)[None, :]
    r = p * RPP + j
    gg = np.empty((128, 2 * RPP), dtype=np.float32)
    gg[:, 0::2] = (r % NG) * STRIDE
    gg[:, 1::2] = (r // NG) * STRIDE
    rt = MAIN + np.arange(TAIL)
    gxt = ((rt % NG) * STRIDE).astype(np.float32)
    gyt = float((MAIN // NG) * STRIDE)  # rows 5760..5775 all have gy=75
    assert np.all(rt // NG == MAIN // NG)
    ident0 = np.eye(NC, dtype=np.float32)
    # stacked square perms: [selA, selB1, selB4, sel42, sel43].  Both
    # split pieces route their real channels to HEAD columns (piece A:
    # cols 0..42, piece B: cols 0..41): with overwrite packing, a later
    # write's junk may only land where junk already was, so real data
    # must sit at the start of each write's column window.
    sels = np.stack(
        [
            _perm_range(85, 0, 43),  # piece A (pairs 1 and 4)
            _perm_range(0, 0, 42),  # piece B pair 1 (t1 rows 0..41)
            _perm_range(1, 0, 42),  # piece B pair 4 (t3 rows 1..42)
            _perm_for(42),  # pair 2
            _perm_for(43),  # pair 5
        ]
    )
    # pack everything into ONE [128, 816] tensor so the const DMA is a
    # single full-128-partition transfer (128 big descriptors) instead of
    # ~870 tiny strided ones that clog the HWDGE descriptor generator:
    # cols [0:640) five 128x128 selectors, [640:730) gg, [730:815) id0
    # (rows 85..127 zero), [815:816) gxt (rows 16..127 zero).
    const = np.zeros((128, 816), dtype=np.float32)
    const[:, 0:640] = sels.transpose(1, 0, 2).reshape(128, 640)
    const[:, 640:730] = gg
    const[:NC, 730:815] = ident0
    const[:TAIL, 815] = gxt
    return gyt, const


GYT_CONST, CONST_TABLE = _tables()
SEL_A, SEL_B1, SEL_B4, SEL_42, SEL_43 = range(5)


def build_program():
    nc = bacc.Bacc(None, target_bir_lowering=False)

    x = nc.dram_tensor("x", (NB, NA * NC, NG, NG), F32, kind="ExternalInput")
    out = nc.dram_tensor("out", (NB, NA * NPOS, NC), F32, kind="ExternalOutput")
    const = nc.dram_tensor("const", (128, 816), F32, kind="ExternalInput")

    with tile.TileContext(nc) as tc:
        with (
            tc.tile_pool(name="constp", bufs=1) as constp,
            tc.tile_pool(name="xp", bufs=1) as xp,
            tc.tile_pool(name="outp", bufs=3) as outp,
            tc.tile_pool(name="pp", bufs=4, space="PSUM") as pp,
            tc.tile_pool(name="tp", bufs=2, space="PSUM") as tp,
        ):
            # one full-128-partition const load on the scalar queue ahead
            # of t2 (a single batch of 128 descriptors, ~1us)
            consts = constp.tile([128, 816], F32)
            nc.scalar.dma_start(out=consts[:], in_=const[:])
            id0s = consts[0:NC, 730:815]
            ggv = consts[:, 640:730].rearrange("p (k c) -> p k c", c=2)
            gxts = consts[0:TAIL, 815:816]

            def sel(i):
                return consts[:, i * 128 : (i + 1) * 128]

            xf = x.rearrange("b c h w -> (b c) (h w)")

            # four full-128-row tiles across three DMA queues; t0 rides
            # the sync queue alone so pair 0's data lands first
            xts = [xp.tile([128, NPOS], F32, name=f"xt{i}") for i in range(4)]
            load_eng = [nc.sync, nc.gpsimd, nc.scalar, nc.gpsimd]
            for i, (r0, r1) in enumerate(TILE_ROWS):
                load_eng[i].dma_start(
                    out=xts[i][0 : r1 - r0, :], in_=xf[r0:r1, :]
                )

            # all six 16-position tails accumulate here; one store at the end
            tta = constp.tile([TAIL, 512], F32)

            # (tile, 45, 128) chunk views: [:, j, :] = chunk j
            def chunks(t, np_):
                return xts[t][0:np_, 0:MAIN].rearrange("c (m j) -> c j m", j=RPP)

            for pair in range(NPAIR):
                b, a = divmod(pair, NA)
                aw = float(ANCHORS[a, 0])
                ah = float(ANCHORS[a, 1])
                src = PAIR_SRC[pair]
                ot = outp.tile([128, RPP * NC + 1], F32, tag="ot")
                otr = ot[:, 0 : RPP * NC].rearrange("p (k c) -> p k c", c=NC)
                tt = tta[:, pair * NC : (pair + 1) * NC]
                pst = tp.tile([TAIL, 512], F32, tag="pst")

                if src[0] == "one" and src[2] == 0:
                    # 85-partition operands, 85x85 identity, 6 chunks/bank
                    t = src[1]
                    xm = chunks(t, NC)
                    for k0, nk in _groups(6):
                        ps = pp.tile([128, 512], F32, tag="ps")
                        for m in range(nk):
                            nc.tensor.transpose(
                                ps[:, NC * m : NC * (m + 1)],
                                xm[:, k0 + m, :],
                                id0s[:, :],
                                tile_position=(0, 0),
                            )
                        nc.scalar.activation(
                            ot[:, k0 * NC : (k0 + nk) * NC],
                            ps[:, 0 : nk * NC],
                            AF.Tanh,
                            scale=0.5,
                        )
                        psv = ps[:, 0 : nk * NC].rearrange("p (k c) -> p k c", c=NC)
                        nc.scalar.activation(
                            otr[:, k0 : k0 + nk, 2:4], psv[:, :, 2:4], AF.Exp
                        )
                    nc.tensor.transpose(
                        pst[:, 0:NC],
                        xts[t][0:NC, MAIN:NPOS],
                        id0s[:, :],
                        tile_position=(0, 0),
                    )
                    nc.scalar.activation(tt, pst[:, 0:NC], AF.Tanh, scale=0.5)
                    nc.scalar.activation(tt[:, 2:4], pst[:, 2:4], AF.Exp)

                elif src[0] == "one":
                    # 128-partition operands + square perm; 128-col writes
                    # at 85-col stride (overwrite packing), 5 chunks/bank
                    t, shift = src[1], src[2]
                    sq = sel(SEL_42 if shift == 42 else SEL_43)
                    xm = chunks(t, 128)
                    for k0, nk in _groups(5):
                        ps = pp.tile([128, 512], F32, tag="ps")
                        for m in range(nk):
                            nc.tensor.transpose(
                                ps[:, NC * m : NC * m + 128],
                                xm[:, k0 + m, :],
                                sq,
                                tile_position=(0, 0),
                            )
                        nc.scalar.activation(
                            ot[:, k0 * NC : (k0 + nk) * NC],
                            ps[:, 0 : nk * NC],
                            AF.Tanh,
                            scale=0.5,
                        )
                        psv = ps[:, 0 : nk * NC].rearrange("p (k c) -> p k c", c=NC)
                        nc.scalar.activation(
                            otr[:, k0 : k0 + nk, 2:4], psv[:, :, 2:4], AF.Exp
                        )
                    nc.tensor.transpose(
                        pst[:, 0:128],
                        xts[t][0:128, MAIN:NPOS],
                        sq,
                        tile_position=(0, 0),
                    )
                    nc.scalar.activation(tt, pst[:, 0:NC], AF.Tanh, scale=0.5)
                    nc.scalar.activation(tt[:, 2:4], pst[:, 2:4], AF.Exp)

                else:
                    # split across two tiles: piece A -> cols 0..42 (43-col
                    # stride), piece B -> cols 43..84 (42-col stride), both
                    # 9 chunks/bank with overwrite packing
                    _, tA, sA, nA, tB, sB = src
                    selA = sel(SEL_A)
                    selB = sel(SEL_B1 if pair == 1 else SEL_B4)
                    xmA = chunks(tA, 128)
                    xmB = chunks(tB, 128)
                    for k0, nk in _groups(9):
                        psA = pp.tile([128, 512], F32, tag="ps")
                        psB = pp.tile([128, 512], F32, tag="ps")
                        for m in range(nk):
                            nc.tensor.transpose(
                                psA[:, nA * m : nA * m + 128],
                                xmA[:, k0 + m, :],
                                selA,
                                tile_position=(0, 0),
                            )
                            nc.tensor.transpose(
                                psB[:, (NC - nA) * m : (NC - nA) * m + 128],
                                xmB[:, k0 + m, :],
                                selB,
                                tile_position=(0, 0),
                            )
                        pvA = psA[:, 0 : nk * nA].rearrange("p (k c) -> p k c", c=nA)
                        pvB = psB[:, 0 : nk * (NC - nA)].rearrange(
                            "p (k c) -> p k c", c=NC - nA
                        )
                        nc.scalar.activation(
                            otr[:, k0 : k0 + nk, 0:nA], pvA, AF.Tanh, scale=0.5
                        )
                        nc.scalar.activation(
                            otr[:, k0 : k0 + nk, nA:NC], pvB, AF.Tanh, scale=0.5
                        )
                        nc.scalar.activation(
                            otr[:, k0 : k0 + nk, 2:4], pvA[:, :, 2:4], AF.Exp
                        )
                    pstB = tp.tile([TAIL, 512], F32, tag="pst")
                    nc.tensor.transpose(
                        pst[:, 0:128],
                        xts[tA][0:128, MAIN:NPOS],
                        selA,
                        tile_position=(0, 0),
                    )
                    nc.tensor.transpose(
                        pstB[:, 0:128],
                        xts[tB][0:128, MAIN:NPOS],
                        selB,
                        tile_position=(0, 0),
                    )
                    nc.scalar.activation(
                        tt[:, 0:nA], pst[:, 0:nA], AF.Tanh, scale=0.5
                    )
                    nc.scalar.activation(
                        tt[:, nA:NC], pstB[:, 0 : NC - nA], AF.Tanh, scale=0.5
                    )
                    nc.scalar.activation(tt[:, 2:4], pst[:, 2:4], AF.Exp)

                # VectorE fixups (main): whole-tile affine at 2x port mode
                # (needs an even element count -> one memset pad column),
                # then per-channel-type corrections.
                nc.vector.memset(ot[:, RPP * NC : RPP * NC + 1], 0.0)
                nc.vector.tensor_scalar(
                    ot[:, 0 : RPP * NC + 1],
                    ot[:, 0 : RPP * NC + 1],
                    0.5,
                    0.5,
                    OP.mult,
                    OP.add,
                )
                xy = otr[:, :, 0:2]
                nc.vector.tensor_scalar(xy, xy, STRIDE, None, OP.mult)
                nc.vector.tensor_tensor(xy, xy, ggv, OP.add)
                wv = otr[:, :, 2:3]
                nc.vector.tensor_scalar(wv, wv, 2.0 * aw, -aw, OP.mult, OP.add)
                hv = otr[:, :, 3:4]
                nc.vector.tensor_scalar(hv, hv, 2.0 * ah, -ah, OP.mult, OP.add)

                # VectorE fixups (tail); odd count (85) -> 84 + last col
                nc.vector.tensor_scalar(
                    tt[:, 0:84], tt[:, 0:84], 0.5, 0.5, OP.mult, OP.add
                )
                nc.vector.tensor_scalar(
                    tt[:, 84:85], tt[:, 84:85], 0.5, 0.5, OP.mult, OP.add
                )
                nc.vector.tensor_scalar(
                    tt[:, 0:1], tt[:, 0:1], STRIDE, gxts[:], OP.mult, OP.add
                )
                nc.vector.tensor_scalar(
                    tt[:, 1:2], tt[:, 1:2], STRIDE, GYT_CONST, OP.mult, OP.add
                )
                nc.vector.tensor_scalar(
                    tt[:, 2:3], tt[:, 2:3], 2.0 * aw, -aw, OP.mult, OP.add
                )
                nc.vector.tensor_scalar(
                    tt[:, 3:4], tt[:, 3:4], 2.0 * ah, -ah, OP.mult, OP.add
                )

                # main store on the sync HWDGE queue: 128 runs of 15.3KB
                obase = a * NPOS
                nc.sync.dma_start(
                    out=out[b, obase : obase + MAIN, :].rearrange(
                        "(p j) c -> p (j c)", p=128
                    ),
                    in_=ot[:, 0 : RPP * NC],
                )

            # one combined tail store: out[b, a*NPOS + 5760 + t, c] with
            # partition t and free (b, a, c) = tta col (b*3+a)*85 + c
            tails = out.rearrange("b (a q) c -> q b a c", a=NA)
            nc.sync.dma_start(
                out=tails[MAIN:NPOS],
                in_=tta[:, 0 : NPAIR * NC].rearrange(
                    "t (b a c) -> t b a c", b=NB, a=NA
                ),
            )

    nc.compile()
    return nc


_NC_CACHE = None


def _get_program():
    global _NC_CACHE
    if _NC_CACHE is None:
        _NC_CACHE = build_program()
    return _NC_CACHE


def run(x, trace=False, **kwargs):
    """x: full (16, 255, 76, 76) f32. Returns (full_out, BassKernelResults)."""
    x = np.ascontiguousarray(np.asarray(x, dtype=np.float32))
    assert x.shape == (NB_FULL, NA * NC, NG, NG), x.shape
    nc = _get_program()
    in_maps = [
        {
            "x": np.ascontiguousarray(x[c * NB : (c + 1) * NB]),
            "const": CONST_TABLE,
        }
        for c in range(N_CORES)
    ]
    res = run_bass_kernel_spmd(nc, in_maps, list(range(N_CORES)), trace=trace, **kwargs)
    out = np.concatenate([res.results[c]["out"] for c in range(N_CORES)], axis=0)
    return out, res


def kernel(x):
    out, _ = run(x, trace=False)
    return out
